# revision 14
# baseline (speedup 1.0000x reference)
"""Trainium2 Bass kernel for nn_LoraAttention (v2 — software-pipelined).

Math (reference): qkv = x@W_qkv.T; lora full proj ql/vl = split(x@W_lora.T + b_lora)
(K-part discarded); low-rank dq = (x@A_q.T)@B_q.T*1/8 (same for v); softmax
attention over H=16 heads, D=64; out = attn_cat@W_out.T + b_out.

Host-side algebra folds every LoRA term into the projection weights:
  Wq_eff = W_qkv[q] + W_lora[q] + (B_q@A_q)/8      (q bias b_lora[q] kept)
  Wk_eff = W_qkv[k]                                 (no bias)
  Wv_eff = W_qkv[v] + W_lora[v] + (B_v@A_v)/8
  v bias commutes through softmax -> host-side b_eff = b_out + W_out @ b_lora[v].

Sharding: 8 cores = 4 batches x 2 head-groups (8 heads each).  Each core
projects QKV for its heads, does attention, and computes a partial output
projection over its 512 concat dims; host sums the two partials per batch.

v2 device schedule (v1 measured 449 us with TensorE 90% busy against a
~273 us matmul-stream floor; the gap was exposed LDWEIGHTS + PE-FIFO stalls):
  - One "slot" = (pair t, query chunk nq, key chunk mq): S^T pair (row-packed
    K=64 matmuls) -> exp (ScalarE, 1024-wide from PSUM) -> PV pair (M=65, the
    65th stationary column = ones puts the softmax denominator in PSUM row 0
    via a leading ones column).
  - PV for slot i is emitted AFTER S+exp of slot i+1, so the PE FIFO never
    parks at a PV waiting on its exp: exp(i) runs while PE does S(i+1)+aux.
  - Projection / output-projection matmuls are sliced into ~2-matmul closures
    and budget-paced into the exp slack of each slot ("aux pump"), with
    force-drain before any consumer is emitted (Tile deps are program-order).
  - PSUM tiles held across closures get an exclusive tag ("gps": KQ groups and
    out-proj groups, which never coexist in the queue) so the pool ring can
    never hand their bank to an interleaved allocation mid-accumulation.
  - Normalization per (t, nq): reciprocal (DVE, direct from PSUM row 0),
    ones-broadcast matmul in fp32r (1 cyc/row vs 4 for fp32), fused
    psum*psum->sbuf bf16 multiply, DMA-pack into acat.  Norm closures live on
    a dedicated stack that is fully drained before the PV accumulators are
    reallocated.
  - PSUM: S double-buffer 2x[128,1024] (4) + gps (1) + pp (1) + atA/atB (2)
    = 8 banks.
"""

import numpy as np
import ml_dtypes
from collections import deque

import concourse.bacc as bacc
import concourse.tile as tile
from concourse import mybir
from concourse.bass_utils import run_bass_kernel_spmd

B, N, C = 4, 2048, 1024
H, D = 16, 64
LORA_SCALE = 1.0 / 8.0
ATTN_SCALE = float(D) ** -0.5  # 0.125

f32 = mybir.dt.float32
f32r = mybir.dt.float32r
bf16 = mybir.dt.bfloat16
BF = ml_dtypes.bfloat16

NQ = 4           # query chunks of 512
MQ = 16          # key chunks of 128
KC = 8           # contraction chunks of 128 over C
PAIRS = 4        # head pairs per core (8 local heads)

_cache: dict = {}


def _build_program():
    nc = bacc.Bacc("TRN2", target_bir_lowering=False, debug=False, num_devices=8)

    xT_d = nc.dram_tensor("xT", [C, N], bf16, kind="ExternalInput").ap()
    wqk_d = nc.dram_tensor("wqk", [C, 1024], bf16, kind="ExternalInput").ap()
    wv_d = nc.dram_tensor("wv", [C, 512], bf16, kind="ExternalInput").ap()
    wo_d = nc.dram_tensor("wo", [512, C], bf16, kind="ExternalInput").ap()
    bq_d = nc.dram_tensor("bq", [128, 4], f32, kind="ExternalInput").ap()
    outT_d = nc.dram_tensor("outT", [C, N], f32, kind="ExternalOutput").ap()

    EXP = mybir.ActivationFunctionType.Exp

    with tile.TileContext(nc) as tc:
        with (
            tc.tile_pool(name="win", bufs=1) as win,        # weights + x + consts
            tc.tile_pool(name="kq", bufs=1) as kqp,         # K/Q bf16 tiles
            tc.tile_pool(name="vp", bufs=1) as vp,          # [1|V] tiles
            tc.tile_pool(name="pex", bufs=5) as pex,        # exp outputs
            tc.tile_pool(name="acat", bufs=1) as acatp,     # normalized attn (d, nq)
            tc.tile_pool(name="scr", bufs=2) as scr,        # small scratch
            tc.tile_pool(name="osb", bufs=3) as osbp,       # out eviction
            tc.tile_pool(name="pp", bufs=1, space="PSUM") as pp,    # 1 bank
            tc.tile_pool(name="gps", bufs=1, space="PSUM") as gps,  # 1 bank
            tc.tile_pool(name="sp", bufs=2, space="PSUM") as spp,   # 4 banks
            tc.tile_pool(name="at", bufs=1, space="PSUM") as atp,   # 2 banks
        ):
            # ---- loads (emission order = consumption order) ----
            xt = [None] * KC
            wqk = [None] * KC
            wv = [None] * KC
            for kc in range(KC):
                xt[kc] = win.tile([128, N], bf16, tag=f"xt{kc}", name=f"xt{kc}")
                nc.sync.dma_start(xt[kc][:], xT_d[kc * 128:(kc + 1) * 128, :])
                wqk[kc] = win.tile([128, 1024], bf16, tag=f"wqk{kc}", name=f"wqk{kc}")
                nc.scalar.dma_start(wqk[kc][:], wqk_d[kc * 128:(kc + 1) * 128, :])
            for kc in range(KC):
                wv[kc] = win.tile([128, 512], bf16, tag=f"wv{kc}", name=f"wv{kc}")
                nc.gpsimd.dma_start(wv[kc][:], wv_d[kc * 128:(kc + 1) * 128, :])
            bqt = win.tile([128, 4], f32, tag="bq")
            nc.gpsimd.dma_start(bqt[:], bq_d[:])
            wo = []
            for dc in range(4):
                t_ = win.tile([128, 1024], bf16, tag=f"wo{dc}", name=f"wo{dc}")
                nc.gpsimd.dma_start(t_[:], wo_d[dc * 128:(dc + 1) * 128, :])
                wo.append(t_)
            warm = win.tile([1, 2], f32, tag="warm")
            nc.vector.memset(warm[0:1, 0:1], 0.0)
            nc.scalar.activation(
                warm[0:1, 1:2], warm[0:1, 0:1],
                mybir.ActivationFunctionType.Exp,
            )

            # ---- aux queue: budget-paced PE filler work ----
            aux = deque()
            nstack = deque()   # norm closures; drained with priority
            debt = [0.0]
            evt: dict = {}

            def pump(budget):
                debt[0] = min(debt[0] + budget, 4000.0)
                while nstack and debt[0] >= nstack[0][0]:
                    cst, fn = nstack.popleft()
                    fn()
                    debt[0] -= cst
                while aux and debt[0] >= aux[0][0]:
                    cst, fn = aux.popleft()
                    fn()
                    debt[0] -= cst

            def drain_norms():
                while nstack:
                    cst, fn = nstack.popleft()
                    fn()

            def force(key):
                while not evt.get(key, False):
                    cst, fn = aux.popleft()
                    fn()

            # ---- KQ projection groups (pair t, kind 0=K/1=Q, token chunk) --
            kq_tiles = {}

            def alloc_pair(t):
                kt = kqp.tile([128, N], bf16, tag=f"k{t % 2}")
                qt = kqp.tile([128, N], bf16, tag=f"q{t % 2}")
                kq_tiles[t] = (kt, qt)

            def kq_step(st, s, t, kind, chunk):
                kt, qt = kq_tiles[t]
                if s == 0:
                    st["ps"] = gps.tile([128, 512], f32, tag="gps", name="gpsb")
                ps = st["ps"]
                off = (512 if kind == 0 else 0) + t * 128
                for kc in (2 * s, 2 * s + 1):
                    nc.tensor.matmul(
                        ps[:],
                        wqk[kc][:, off:off + 128],
                        xt[kc][:, chunk * 512:(chunk + 1) * 512],
                        start=(kc == 0), stop=(kc == KC - 1),
                    )
                if s == 3:
                    sl = slice(chunk * 512, (chunk + 1) * 512)
                    if kind == 0:
                        nc.vector.tensor_copy(kt[:, sl], ps[:])
                    else:
                        nc.vector.tensor_scalar_add(qt[:, sl], ps[:], bqt[:, t:t + 1])
                    evt[(t, kind, chunk)] = True

            def push_kq_group(t, kind, chunk):
                st = {}
                aux.extend(
                    (460.0, (lambda s: (lambda: kq_step(st, s, t, kind, chunk)))(s))
                    for s in range(4)
                )

            def run_kq_group(t, kind, chunk):
                st = {}
                for s in range(4):
                    kq_step(st, s, t, kind, chunk)

            # ---- V projection (all 8 heads, ones in column 0) ----
            vts = [None] * MQ

            def v_proj(mq):
                vt = vp.tile([128, 8, 65], bf16, tag=f"v{mq}")
                nc.vector.memset(vt[:, :, 0:1], 1.0)
                ps = pp.tile([128, 512], f32, tag="pp")
                for kc in range(KC):
                    nc.tensor.matmul(
                        ps[:], xt[kc][:, mq * 128:(mq + 1) * 128], wv[kc][:],
                        start=(kc == 0), stop=(kc == KC - 1),
                    )
                nc.vector.tensor_copy(
                    vt[:, :, 1:65], ps[:].rearrange("p (h e) -> p h e", h=8)
                )
                vts[mq] = vt

            # ---- normalization + out-projection ----
            acat = [[None] * PAIRS for _ in range(NQ)]

            def norm_half(t, nq, ar32, half):
                ac = acat[nq][t]
                rr = scr.tile([1, 512], f32, tag=f"rr{half}")
                nc.vector.reciprocal_approx_fast(rr[0:1, :], ar32[0:1, :])
                rbs = scr.tile([65, 512], f32, tag=f"rbs{half}")
                nc.gpsimd.partition_broadcast(rbs[:], rr[:])
                acn = scr.tile([65, 512], bf16, tag=f"acn{half}")
                nc.vector.tensor_mul(acn[:], ar32[:], rbs[:])
                nc.sync.dma_start(ac[half * 64:(half + 1) * 64, :], acn[1:65, :])

            def emit_norm(t, nq, atA, atB, final=False):
                ac = acatp.tile([128, 512], bf16, tag=f"ac{nq}_{t}")
                acat[nq][t] = ac
                for at, half in ((atA, 0), (atB, 1)):
                    if final:
                        src_ap = at
                    else:
                        ar32 = scr.tile([65, 512], f32, tag=f"ar{half}")
                        nc.vector.tensor_copy(ar32[:], at[:])
                        src_ap = ar32
                    nstack.append(
                        (150.0,
                         (lambda a=src_ap, h=half: norm_half(t, nq, a, h)))
                    )

            def out_step(st, dc, nq, cc):
                if dc == 0:
                    pool = gps if cc % 2 == 0 else pp
                    st["ps"] = pool.tile(
                        [128, 512], f32, tag=pool is gps and "gps" or "pp",
                        name="outps",
                    )
                ps = st["ps"]
                nc.tensor.matmul(
                    ps[:],
                    wo[dc][:, cc * 128:(cc + 1) * 128],
                    acat[nq][dc][:],
                    start=(dc == 0), stop=(dc == 3),
                )
                if dc == 3:
                    ob = osbp.tile([128, 512], f32, tag="ob")
                    nc.vector.tensor_copy(ob[:], ps[:])
                    nc.sync.dma_start(
                        outT_d[cc * 128:(cc + 1) * 128, nq * 512:(nq + 1) * 512],
                        ob[:],
                    )

            def push_out(nq, final=False):
                if not final:
                    for cc in range(8):
                        st = {}
                        aux.extend(
                            (240.0,
                             (lambda st=st, dc=dc, cc=cc: out_step(st, dc, nq, cc)))
                            for dc in range(4)
                        )
                    return
                # final wave: keep PE busy during the last norm chain by
                # running each group's pair-0..2 accumulations first; the
                # pair-3 matmul (+ eviction) follows two groups later
                sts = [dict() for _ in range(8)]
                def d012(cc):
                    for dc in range(3):
                        out_step(sts[cc], dc, nq, cc)
                def d3(cc):
                    out_step(sts[cc], 3, nq, cc)
                aux.append((700.0, lambda: d012(0)))
                aux.append((700.0, lambda: d012(1)))
                for cc in range(2, 8):
                    aux.append((240.0, (lambda c=cc: d3(c - 2))))
                    aux.append((700.0, (lambda c=cc: d012(c))))
                aux.append((240.0, lambda: d3(6)))
                aux.append((240.0, lambda: d3(7)))

            # ---- pending-PV pipeline ----
            pending = [None]
            at_tiles = {}

            def flush():
                if pending[0] is None:
                    return
                pe, ft, fnq, fmq = pending[0]
                pending[0] = None
                if fmq == 0:
                    drain_norms()   # old atA/atB consumers must be emitted first
                    at_tiles[(ft, fnq)] = (
                        atp.tile([65, 512], f32, tag="atA", name="atA"),
                        atp.tile([65, 512], f32, tag="atB", name="atB"),
                    )
                atA, atB = at_tiles[(ft, fnq)]
                nc.tensor.matmul(
                    atA[:], vts[fmq][:, 2 * ft, :], pe[:, 0:512],
                    start=(fmq == 0), stop=(fmq == MQ - 1),
                )
                nc.tensor.matmul(
                    atB[:], vts[fmq][:, 2 * ft + 1, :], pe[:, 512:1024],
                    start=(fmq == 0), stop=(fmq == MQ - 1),
                )
                if fmq == MQ - 1:
                    emit_norm(ft, fnq, atA, atB,
                              final=(ft == PAIRS - 1 and fnq == NQ - 1))
                    del at_tiles[(ft, fnq)]
                    if ft == PAIRS - 1:
                        push_out(fnq, final=(fnq == NQ - 1))

            # ---- pre-phase: pair-0 first chunks + first V tiles,
            # interleaved kc-major so the PE keeps pace with the load stream
            alloc_pair(0)
            kt0, qt0 = kq_tiles[0]
            vt0 = vp.tile([128, 8, 65], bf16, tag="v0", name="vt0")
            vt1 = vp.tile([128, 8, 65], bf16, tag="v1", name="vt1")
            nc.vector.memset(vt0[:, :, 0:1], 1.0)
            nc.vector.memset(vt1[:, :, 0:1], 1.0)
            vts[0], vts[1] = vt0, vt1
            ps_k = gps.tile([128, 512], f32, tag="gps", name="gpsb")
            ps_q = pp.tile([128, 512], f32, tag="pp", name="ppb")
            sp_pre = spp.tile([128, 1024], f32, tag="sp", name="sp_pre")
            for kc in range(KC):
                nc.tensor.matmul(
                    ps_k[:], wqk[kc][:, 512:640], xt[kc][:, 0:512],
                    start=(kc == 0), stop=(kc == KC - 1),
                )
                nc.tensor.matmul(
                    ps_q[:], wqk[kc][:, 0:128], xt[kc][:, 0:512],
                    start=(kc == 0), stop=(kc == KC - 1),
                )
                nc.tensor.matmul(
                    sp_pre[:, 0:512], xt[kc][:, 0:128], wv[kc][:],
                    start=(kc == 0), stop=(kc == KC - 1),
                )
                nc.tensor.matmul(
                    sp_pre[:, 512:1024], xt[kc][:, 128:256], wv[kc][:],
                    start=(kc == 0), stop=(kc == KC - 1),
                )
            nc.vector.tensor_copy(kt0[:, 0:512], ps_k[:])
            nc.vector.tensor_scalar_add(qt0[:, 0:512], ps_q[:], bqt[:, 0:1])
            nc.vector.tensor_copy(
                vt0[:, :, 1:65],
                sp_pre[:, 0:512].rearrange("p (h e) -> p h e", h=8),
            )
            nc.vector.tensor_copy(
                vt1[:, :, 1:65],
                sp_pre[:, 512:1024].rearrange("p (h e) -> p h e", h=8),
            )
            evt[(0, 0, 0)] = True
            evt[(0, 1, 0)] = True
            for chunk in (1, 2, 3):
                push_kq_group(0, 0, chunk)
            for chunk in (1, 2, 3):
                push_kq_group(0, 1, chunk)

            # ---- main loop ----
            for t in range(PAIRS):
                kt, qt = kq_tiles[t]
                for nq in range(NQ):
                    force((t, 1, nq))
                    for mq in range(MQ):
                        if nq == 0 and mq == 8 and t + 1 < PAIRS:
                            alloc_pair(t + 1)
                            for chunk in range(4):
                                push_kq_group(t + 1, 0, chunk)
                            for chunk in range(4):
                                push_kq_group(t + 1, 1, chunk)
                        if mq % 4 == 0:
                            force((t, 0, mq // 4))
                        # S pair for this slot
                        sp = spp.tile([128, 1024], f32, tag="sp")
                        nc.tensor.matmul(
                            sp[:, 0:512],
                            kt[0:64, mq * 128:(mq + 1) * 128],
                            qt[0:64, nq * 512:(nq + 1) * 512],
                            start=True, stop=True, tile_position=(0, 0),
                        )
                        nc.tensor.matmul(
                            sp[:, 512:1024],
                            kt[64:128, mq * 128:(mq + 1) * 128],
                            qt[64:128, nq * 512:(nq + 1) * 512],
                            start=True, stop=True, tile_position=(64, 0),
                        )
                        pe = pex.tile([128, 1024], bf16, tag="pe")
                        nc.scalar.activation(pe[:], sp[:], EXP, scale=ATTN_SCALE)
                        # mandatory early V tiles (2 slots ahead of first use)
                        if t == 0 and nq == 0 and mq <= 13:
                            v_proj(mq + 2)
                        # paced filler work
                        if t == 0 and nq == 0:
                            budget = 700.0
                        elif t < 3:
                            budget = 290.0
                        else:
                            budget = 560.0
                        # at a block boundary, flush first: the previous
                        # block's last PV + at-freeing cast reach their queues
                        # ahead of pumped filler evictions
                        if mq == 0:
                            flush()
                            pump(budget)
                        else:
                            pump(budget)
                            flush()
                        pending[0] = (pe, t, nq, mq)

            flush()
            # keep PE busy through the final norm chain: the first out-proj
            # groups (dc 0-2 accumulations) have no dependency on it
            for _ in range(2):
                if aux:
                    cst, fn = aux.popleft()
                    fn()
            drain_norms()
            while aux:
                cst, fn = aux.popleft()
                fn()
            drain_norms()

    nc.compile()
    return nc


def _get_program():
    if "nc" not in _cache:
        _cache["nc"] = _build_program()
    return _cache["nc"]


def _prep_in_maps(x, W_qkv, W_lora, b_lora, A_q, B_q, A_v, B_v, W_out):
    HD = H * D  # 1024
    Wq = W_qkv[0:HD] + W_lora[0:HD] + LORA_SCALE * (B_q @ A_q)
    Wk = W_qkv[HD:2 * HD]
    Wv = W_qkv[2 * HD:3 * HD] + W_lora[2 * HD:3 * HD] + LORA_SCALE * (B_v @ A_v)
    bq = b_lora[0:HD]

    xT = [np.ascontiguousarray(x[b].T).astype(BF) for b in range(B)]
    in_maps = []
    for c in range(8):
        b, hg = divmod(c, 2)
        sel = slice(hg * 512, (hg + 1) * 512)
        wqk_c = np.ascontiguousarray(
            np.concatenate([Wq[sel], Wk[sel]], axis=0).T
        ).astype(BF)
        wv_c = np.ascontiguousarray(Wv[sel].T).astype(BF)
        wo_c = np.ascontiguousarray(W_out[:, sel].T).astype(BF)
        bq_c = np.ascontiguousarray(bq[sel].reshape(4, 128).T).astype(np.float32)
        in_maps.append({
            "xT": xT[b], "wqk": wqk_c, "wv": wv_c, "wo": wo_c, "bq": bq_c,
        })
    return in_maps


def kernel(x, W_qkv, W_lora, b_lora, A_q, B_q, A_v, B_v, W_out, b_out):
    x = np.asarray(x, np.float32)
    W_qkv = np.asarray(W_qkv, np.float32)
    W_lora = np.asarray(W_lora, np.float32)
    b_lora = np.asarray(b_lora, np.float32)
    A_q = np.asarray(A_q, np.float32)
    B_q = np.asarray(B_q, np.float32)
    A_v = np.asarray(A_v, np.float32)
    B_v = np.asarray(B_v, np.float32)
    W_out = np.asarray(W_out, np.float32)
    b_out = np.asarray(b_out, np.float32)

    in_maps = _prep_in_maps(x, W_qkv, W_lora, b_lora, A_q, B_q, A_v, B_v, W_out)
    b_eff = b_out + W_out @ b_lora[2 * H * D:3 * H * D]

    nc = _get_program()
    res = run_bass_kernel_spmd(nc, in_maps, list(range(8)))

    out = np.empty((B, N, C), np.float32)
    for b in range(B):
        acc = res.results[2 * b]["outT"] + res.results[2 * b + 1]["outT"]
        acc += b_eff[:, None]
        out[b] = acc.T
    return out
